# revision 4
# baseline (speedup 1.0000x reference)
"""Causal self-attention (B=16, S=2048, D=512) on 8 Trainium2 NeuronCores.

Strategy: data-parallel over batch (2 sequences per core), QKV weights
replicated and persistent in SBUF. All PE operands are bf16 (full-rate on
the 128x128 array, fp32 PSUM accumulation); evictions/output are fp32
where the contract requires it.

  host prep:  xT = x^T per sequence [D, S] bf16;  wqT = Wq^T/sqrt(D) bf16;
              wkT = Wk^T bf16; wvT = Wv^T bf16; bq' = bq/sqrt(D) f32;
              bvb = bv broadcast [128, D] f32; key-pad bias (0/-1e30) f32;
              query mask (1/0) f32; causal triangular tiles bf16;
              identity + ones column bf16.

  device (per sequence, cap c = valid 128-blocks):
    QT[d,s] = wqT^T.slices @ xT   (+bq' on DVE eviction, bf16 out)
    KT[d,s] = wkT^T.slices @ xT   (+bk on ACT Copy eviction, bf16 out)
    V[s,d]  = xT^T.slices @ wvT   (+bv broadcast on DVE eviction, bf16)
    per q-chunk (512 queries):
      scoresT[k,q] = KT.T @ QT    (diagonal blocks get the triangular
                                   -1e30 tile pre-added via ident@causal
                                   as the start=True matmul of the group)
      expT = Exp(scoresT + keybias[k])   (ACT, bf16 out; no max-sub
                                          needed: |scores| <~ 40)
      dacc = bf16 pairwise tree-sum of expT tiles     (DVE)
      out_un[q,d] = expT.slices^T @ V    (PE, accumulate over k <= q)
      denom[q]    = dacc.slices^T @ ones (4x N=1 matmuls -> [128q, 4],
                                          no transpose round-trip)
      out = out_un * (recip(denom+eps) * qmask)[q]    (DVE per-partition)

PE program order per chunk is scores -> AV -> denom so the tensor engine
never waits on the softmax-scale path.
"""

import numpy as np
import ml_dtypes

import concourse.bacc as bacc
import concourse.mybir as mybir
from concourse.tile import TileContext
from concourse.bass_utils import run_bass_kernel_spmd

B, S, D = 16, 2048, 512
N_CORES = 8
BPC = B // N_CORES          # sequences per core
P = 128                     # partition dim
W = 512                     # matmul moving width (one PSUM bank of fp32)
DC = D // P                 # 4 contraction chunks of 128 over D
SB = S // P                 # 16 blocks of 128 over S
NEG = -1.0e30
EPS = 1.0e-30

f32 = mybir.dt.float32
bf16 = mybir.dt.bfloat16
BF16 = ml_dtypes.bfloat16


def build_nc(repeat: int = 1, loop: bool = False, slot_caps=(SB, SB)):
    """slot_caps[s] = valid (non-padded) 128-blocks for sequence slot s on
    every core (program-wide max). Blocks beyond the cap hold only padded
    positions: key bias zeroes their keys, qmask zeroes their query rows,
    and their output blocks are zero-filled — exact for any mask."""
    nc = bacc.Bacc()

    xT_d = nc.declare_dram_parameter("xT", [BPC, D, S], bf16, isOutput=False)
    wqT_d = nc.declare_dram_parameter("wqT", [D, D], bf16, isOutput=False)
    wkT_d = nc.declare_dram_parameter("wkT", [D, D], bf16, isOutput=False)
    wvT_d = nc.declare_dram_parameter("wvT", [D, D], bf16, isOutput=False)
    bq_d = nc.declare_dram_parameter("bq", [D], f32, isOutput=False)
    bk_d = nc.declare_dram_parameter("bk", [D], f32, isOutput=False)
    bvb_d = nc.declare_dram_parameter("bvb", [P, D], f32, isOutput=False)
    kbias_d = nc.declare_dram_parameter("kbias", [BPC, S], f32, isOutput=False)
    qmask_d = nc.declare_dram_parameter("qmask", [BPC, S], f32, isOutput=False)
    causal_d = nc.declare_dram_parameter("causal", [W // P, P, W], bf16,
                                         isOutput=False)
    ident_d = nc.declare_dram_parameter("ident", [P, P], bf16, isOutput=False)
    ones_d = nc.declare_dram_parameter("ones", [P, 1], bf16, isOutput=False)
    out_d = nc.declare_dram_parameter("out", [BPC, S, D], f32, isOutput=True)

    with TileContext(nc) as tc:
        with (
            tc.tile_pool(name="persist", bufs=1) as pers,
            tc.tile_pool(name="xt", bufs=2 * DC) as xp,
            tc.tile_pool(name="qt", bufs=DC) as qp,
            tc.tile_pool(name="kt", bufs=DC) as kp,
            tc.tile_pool(name="vv", bufs=SB) as vp,
            tc.tile_pool(name="ex", bufs=SB + 2) as ep,
            tc.tile_pool(name="da", bufs=SB) as dap,
            tc.tile_pool(name="outs", bufs=4) as op_,
            tc.tile_pool(name="misc", bufs=8) as mp,
            tc.tile_pool(name="pqs", bufs=3, space="PSUM") as pqs,
            tc.tile_pool(name="po", bufs=4, space="PSUM") as pop,
            tc.tile_pool(name="pd", bufs=1, space="PSUM") as pdp,
        ):
            # ---- persistent setup (once) ----
            wq, wk, wv = [], [], []
            for c in range(DC):
                t = pers.tile([P, W], bf16, tag=f"wq{c}")
                nc.sync.dma_start(out=t[:], in_=wqT_d[c * P:(c + 1) * P, :])
                wq.append(t)
            for c in range(DC):
                t = pers.tile([P, W], bf16, tag=f"wk{c}")
                nc.sync.dma_start(out=t[:], in_=wkT_d[c * P:(c + 1) * P, :])
                wk.append(t)
            for c in range(DC):
                t = pers.tile([P, W], bf16, tag=f"wv{c}")
                nc.sync.dma_start(out=t[:], in_=wvT_d[c * P:(c + 1) * P, :])
                wv.append(t)
            bq_t = pers.tile([P, DC], f32, tag="bq")
            nc.sync.dma_start(out=bq_t[:], in_=bq_d.rearrange("(n p) -> p n", p=P))
            bk_t = pers.tile([P, DC], f32, tag="bk")
            nc.sync.dma_start(out=bk_t[:], in_=bk_d.rearrange("(n p) -> p n", p=P))
            bvb_t = pers.tile([P, W], f32, tag="bvb")
            nc.sync.dma_start(out=bvb_t[:], in_=bvb_d[:])
            causal = []
            for j in range(W // P):
                t = pers.tile([P, W], bf16, tag=f"causal{j}")
                nc.sync.dma_start(out=t[:], in_=causal_d[j])
                causal.append(t)
            ident_t = pers.tile([P, P], bf16, tag="ident")
            nc.sync.dma_start(out=ident_t[:], in_=ident_d[:])
            ones_t = pers.tile([P, 1], bf16, tag="ones")
            nc.sync.dma_start(out=ones_t[:], in_=ones_d[:])
            zt = pers.tile([P, W], f32, tag="zt")
            nc.gpsimd.memset(zt[:], 0.0)

            import contextlib
            rep_ctx = (
                tc.For_i(0, repeat, 1) if loop else contextlib.nullcontext(0)
            )
            with rep_ctx:
              for _rep in range(1 if loop else repeat):
                for seq in range(BPC):
                    KB = slot_caps[seq]
                    SCcap = -(-KB // (W // P))
                    kbias_t = mp.tile([P, SB], f32, tag="kbias")
                    nc.sync.dma_start(
                        out=kbias_t[:],
                        in_=kbias_d[seq].rearrange("(n p) -> p n", p=P),
                    )
                    qmask_t = mp.tile([P, SB], f32, tag="qmask")
                    nc.sync.dma_start(
                        out=qmask_t[:],
                        in_=qmask_d[seq].rearrange("(n p) -> p n", p=P),
                    )
                    xt = []
                    for c in range(DC):
                        t = xp.tile([P, S], bf16, tag="xt")
                        nc.sync.dma_start(
                            out=t[:], in_=xT_d[seq, c * P:(c + 1) * P, :]
                        )
                        xt.append(t)

                    # ---- Q/K projections (exact widths up to the cap) ----
                    qT = [qp.tile([P, S], bf16, name="qt", tag="qt")
                          for _ in range(DC)]
                    kT = [kp.tile([P, S], bf16, name="kt", tag="kt")
                          for _ in range(DC)]
                    for db in range(DC):
                        for sc in range(SCcap):
                            w = min(W, KB * P - sc * W)
                            pq = pqs.tile([P, W], f32, tag="pqs")
                            for c in range(DC):
                                nc.tensor.matmul(
                                    pq[:, :w],
                                    wq[c][:, db * P:(db + 1) * P],
                                    xt[c][:, sc * W:sc * W + w],
                                    start=(c == 0),
                                    stop=(c == DC - 1),
                                )
                            nc.vector.tensor_scalar_add(
                                qT[db][:, sc * W:sc * W + w],
                                pq[:, :w],
                                bq_t[:, db:db + 1],
                            )
                            pk = pqs.tile([P, W], f32, tag="pqs")
                            for c in range(DC):
                                nc.tensor.matmul(
                                    pk[:, :w],
                                    wk[c][:, db * P:(db + 1) * P],
                                    xt[c][:, sc * W:sc * W + w],
                                    start=(c == 0),
                                    stop=(c == DC - 1),
                                )
                            nc.scalar.activation(
                                kT[db][:, sc * W:sc * W + w],
                                pk[:, :w],
                                mybir.ActivationFunctionType.Identity,
                                bias=bk_t[:, db:db + 1],
                                scale=1.0,
                            )

                    # ---- V projection (bias via DVE broadcast add) ----
                    vv = []
                    for sb in range(KB):
                        pv = pqs.tile([P, W], f32, tag="pqs")
                        for c in range(DC):
                            nc.tensor.matmul(
                                pv[:],
                                xt[c][:, sb * P:(sb + 1) * P],
                                wv[c][:],
                                start=(c == 0),
                                stop=(c == DC - 1),
                            )
                        tv = vp.tile([P, W], bf16, tag="vv")
                        nc.vector.tensor_add(tv[:], pv[:], bvb_t[:])
                        vv.append(tv)

                    # ---- attention, one q-chunk (512 queries) at a time ----
                    for qc in range(SCcap):
                        kmax = min((qc + 1) * (W // P), KB)
                        exps = []
                        for kb in range(kmax):
                            ps = pqs.tile([P, W], f32, tag="pqs")
                            j = kb - qc * (W // P)
                            first = True
                            if j >= 0:
                                # diagonal block: seed PSUM with -1e30 tri
                                nc.tensor.matmul(
                                    ps[:], ident_t[:], causal[j][:],
                                    start=True, stop=False,
                                )
                                first = False
                            for dc in range(DC):
                                nc.tensor.matmul(
                                    ps[:],
                                    kT[dc][:, kb * P:(kb + 1) * P],
                                    qT[dc][:, qc * W:(qc + 1) * W],
                                    start=(first and dc == 0),
                                    stop=(dc == DC - 1),
                                )
                            et = ep.tile([P, W], bf16, tag="ex")
                            nc.scalar.activation(
                                et[:],
                                ps[:],
                                mybir.ActivationFunctionType.Exp,
                                bias=kbias_t[:, kb:kb + 1],
                                scale=1.0,
                            )
                            exps.append(et)

                        # denominator: bf16 pairwise tree-sum on DVE
                        level = exps
                        while len(level) > 1:
                            nxt = []
                            i = 0
                            while i + 1 < len(level):
                                t = dap.tile([P, W], bf16, tag="da")
                                nc.vector.tensor_add(
                                    t[:], level[i][:], level[i + 1][:]
                                )
                                nxt.append(t)
                                i += 2
                            if i < len(level):
                                nxt.append(level[i])
                            level = nxt
                        dacc = level[0]

                        # out_un[q,d] = sum_k expT[k,q]^T V[k,d]
                        po_list = []
                        for jq in range(W // P):
                            qb = qc * (W // P) + jq
                            if qb >= KB:
                                continue
                            po = pop.tile([P, W], f32, tag="po")
                            for kb in range(qb + 1):
                                nc.tensor.matmul(
                                    po[:],
                                    exps[kb][:, jq * P:(jq + 1) * P],
                                    vv[kb][:],
                                    start=(kb == 0),
                                    stop=(kb == qb),
                                )
                            po_list.append((jq, qb, po))

                        # denom straight into [128q, 4] layout: 4x N=1 mm
                        pd = pdp.tile([P, W // P], f32, tag="pd")
                        for j in range(W // P):
                            nc.tensor.matmul(
                                pd[:, j:j + 1],
                                dacc[:, j * P:(j + 1) * P],
                                ones_t[:],
                                start=True, stop=True,
                            )
                        scl = mp.tile([P, W // P], f32, tag="scl")
                        nc.vector.tensor_scalar_add(scl[:], pd[:], EPS)
                        nc.vector.reciprocal(scl[:], scl[:])
                        nc.vector.tensor_tensor(
                            scl[:],
                            scl[:],
                            qmask_t[:, qc * (W // P):(qc + 1) * (W // P)],
                            op=mybir.AluOpType.mult,
                        )
                        for jq, qb, po in po_list:
                            ot = op_.tile([P, W], f32, tag="outs")
                            nc.vector.tensor_scalar_mul(
                                ot[:], po[:], scl[:, jq:jq + 1]
                            )
                            nc.sync.dma_start(
                                out=out_d[seq, qb * P:(qb + 1) * P, :],
                                in_=ot[:],
                            )

                    # rows in blocks >= KB are entirely padded queries: zero
                    for qb in range(KB, SB):
                        nc.sync.dma_start(
                            out=out_d[seq, qb * P:(qb + 1) * P, :],
                            in_=zt[:],
                        )
    nc.finalize()
    return nc


def prep_inputs(x, Wq, bq, Wk, bk, Wv, bv, padding_mask):
    """Host-side layout prep + sharding. Returns per-core in_maps."""
    x = np.asarray(x, dtype=np.float32)
    pad = np.asarray(padding_mask).astype(bool)
    sc = 1.0 / np.sqrt(np.float32(D))
    wqT = np.ascontiguousarray((np.asarray(Wq, np.float32).T * sc)).astype(BF16)
    wkT = np.ascontiguousarray(np.asarray(Wk, np.float32).T).astype(BF16)
    wvT = np.ascontiguousarray(np.asarray(Wv, np.float32).T).astype(BF16)
    bq_s = (np.asarray(bq, np.float32) * sc).astype(np.float32)
    bk_a = np.asarray(bk, np.float32)
    bvb = np.tile(np.asarray(bv, np.float32).reshape(1, D), (P, 1))
    kbias = np.where(pad, np.float32(NEG), np.float32(0.0)).astype(np.float32)
    qmask = np.where(pad, np.float32(0.0), np.float32(1.0)).astype(np.float32)

    # triangular -1e30 tiles for the 4 diagonal sub-blocks of a
    # [k=128, q=512] scoresT tile: mask where 128*j + k_local > q_local
    kl = np.arange(P)[:, None]
    ql = np.arange(W)[None, :]
    causal = np.stack(
        [np.where(P * j + kl > ql, np.float32(NEG), np.float32(0.0))
         for j in range(W // P)]
    ).astype(BF16)
    ident = np.eye(P, dtype=np.float32).astype(BF16)
    ones = np.ones((P, 1), dtype=np.float32).astype(BF16)

    xT = np.ascontiguousarray(x.transpose(0, 2, 1)).astype(BF16)  # [B, D, S]

    # per-seq valid-block cap from the actual mask (exact for any mask)
    valid = ~pad
    caps = np.zeros(B, dtype=np.int64)
    for b in range(B):
        idx = np.nonzero(valid[b])[0]
        caps[b] = 0 if idx.size == 0 else int(np.ceil((idx[-1] + 1) / P))
    order = np.argsort(-caps, kind="stable")  # descending cap
    perm = []
    for i in range(N_CORES):
        perm.extend([int(order[B - 1 - i]), int(order[i])])
    slot_caps = (int(caps[order[N_CORES]]), int(caps[order[0]]))

    in_maps = []
    for i in range(N_CORES):
        sel = [perm[2 * i], perm[2 * i + 1]]
        in_maps.append({
            "xT": np.ascontiguousarray(xT[sel]),
            "wqT": wqT, "wkT": wkT, "wvT": wvT,
            "bq": bq_s, "bk": bk_a, "bvb": bvb,
            "kbias": np.ascontiguousarray(kbias[sel]),
            "qmask": np.ascontiguousarray(qmask[sel]),
            "causal": causal, "ident": ident, "ones": ones,
        })
    return in_maps, perm, slot_caps


_NC_CACHE = {}


def get_nc(repeat: int = 1, loop: bool = False, slot_caps=(SB, SB)):
    key = (repeat, loop, slot_caps)
    if key not in _NC_CACHE:
        _NC_CACHE[key] = build_nc(repeat, loop, slot_caps)
    return _NC_CACHE[key]


def kernel(x, Wq, bq, Wk, bk, Wv, bv, padding_mask):
    in_maps, perm, slot_caps = prep_inputs(
        x, Wq, bq, Wk, bk, Wv, bv, padding_mask)
    nc = get_nc(1, slot_caps=slot_caps)
    r = run_bass_kernel_spmd(nc, in_maps, list(range(N_CORES)))
    out = np.empty((B, S, D), dtype=np.float32)
    for j, orig in enumerate(perm):
        out[orig] = r.results[j // BPC]["out"][j % BPC]
    return out


# revision 11
# speedup vs baseline: 1.1203x; 1.1203x over previous
"""Causal self-attention (B=16, S=2048, D=512) on 8 Trainium2 NeuronCores.

Strategy: data-parallel over batch (2 sequences per core), QKV weights
replicated and persistent in SBUF. All PE operands are bf16 (full-rate on
the 128x128 array, fp32 PSUM accumulation); evictions/output are fp32
where the contract requires it.

  host prep:  xT = x^T per sequence [D, S] bf16;  wqT = Wq^T/sqrt(D) bf16;
              wkT = Wk^T bf16; wvT = Wv^T bf16; bq' = bq/sqrt(D) f32;
              bvb = bv broadcast [128, D] f32; key-pad bias (0/-1e30) f32;
              query mask (1/0) f32; causal triangular tiles bf16;
              identity + ones column bf16.

  device (per sequence, cap c = valid 128-blocks):
    QT[d,s] = wqT^T.slices @ xT   (+bq' on ACT Identity eviction, bf16)
    KT[d,s] = wkT^T.slices @ xT   (+bk on ACT Identity eviction, bf16)
    V[s,d]  = xT^T.slices @ wvT   (+bv broadcast on DVE eviction, bf16)
    per q-chunk (512 queries):
      scoresT[k,q] = KT.T @ QT
      expT = Exp(scoresT + keybias[k])   (ACT, bf16 out; no max-sub
                                          needed: |scores| <~ 40)
      diagonal blocks: expT *= 0/1 triangle  (DVE bf16 2x)
      dacc = bf16 pairwise tree-sum of expT tiles     (DVE)
      out_un[q,d] = expT.slices^T @ V    (PE, accumulate over k <= q)
      denom[q]    = dacc.slices^T @ ones (4x N=1 matmuls -> [128q, 4],
                                          no transpose round-trip)
      out = out_un * (recip(denom+eps) * qmask)[q]    (DVE per-partition)

PE program order per chunk is scores -> AV -> denom so the tensor engine
never waits on the softmax-scale path.
"""

import numpy as np
import ml_dtypes

import concourse.bacc as bacc
import concourse.mybir as mybir
from concourse.tile import TileContext
from concourse.bass_utils import run_bass_kernel_spmd

B, S, D = 16, 2048, 512
N_CORES = 8
BPC = B // N_CORES          # sequences per core
P = 128                     # partition dim
W = 512                     # matmul moving width (one PSUM bank of fp32)
DC = D // P                 # 4 contraction chunks of 128 over D
SB = S // P                 # 16 blocks of 128 over S
NEG = -1.0e30
EPS = 1.0e-30

f32 = mybir.dt.float32
bf16 = mybir.dt.bfloat16
BF16 = ml_dtypes.bfloat16


def build_nc(repeat: int = 1, loop: bool = False, slot_caps=(SB, SB)):
    """slot_caps[s] = valid (non-padded) 128-blocks for sequence slot s on
    every core (program-wide max). Blocks beyond the cap hold only padded
    positions: key bias zeroes their keys, qmask zeroes their query rows,
    and their output blocks are zero-filled — exact for any mask."""
    nc = bacc.Bacc()

    xT_d = nc.declare_dram_parameter("xT", [BPC, D, S], bf16, isOutput=False)
    wqT_d = nc.declare_dram_parameter("wqT", [D, D], bf16, isOutput=False)
    wkT_d = nc.declare_dram_parameter("wkT", [D, D], bf16, isOutput=False)
    wvT_d = nc.declare_dram_parameter("wvT", [D, D], bf16, isOutput=False)
    bq_d = nc.declare_dram_parameter("bq", [D], f32, isOutput=False)
    bk_d = nc.declare_dram_parameter("bk", [D], f32, isOutput=False)
    bvb_d = nc.declare_dram_parameter("bvb", [P, D], f32, isOutput=False)
    kbias_d = nc.declare_dram_parameter("kbias", [BPC, S], f32, isOutput=False)
    qmask_d = nc.declare_dram_parameter("qmask", [BPC, S], f32, isOutput=False)
    causal_d = nc.declare_dram_parameter("causal", [W // P, P, W], bf16,
                                         isOutput=False)
    ones_d = nc.declare_dram_parameter("ones", [P, 1], bf16, isOutput=False)
    out_d = nc.declare_dram_parameter("out", [BPC, S, D], f32, isOutput=True)

    with TileContext(nc) as tc:
        with (
            tc.tile_pool(name="persist", bufs=1) as pers,
            tc.tile_pool(name="xt", bufs=2 * DC) as xp,
            tc.tile_pool(name="qt", bufs=DC) as qp,
            tc.tile_pool(name="kt", bufs=DC) as kp,
            tc.tile_pool(name="vv", bufs=SB) as vp,
            tc.tile_pool(name="ex", bufs=SB + 2) as ep,
            tc.tile_pool(name="da", bufs=SB) as dap,
            tc.tile_pool(name="outs", bufs=4) as op_,
            tc.tile_pool(name="misc", bufs=8) as mp,
            tc.tile_pool(name="pqs", bufs=3, space="PSUM") as pqs,
            tc.tile_pool(name="po", bufs=4, space="PSUM") as pop,
            tc.tile_pool(name="pd", bufs=1, space="PSUM") as pdp,
        ):
            # ---- persistent setup (once) ----
            wq, wk, wv = [], [], []
            for c in range(DC):
                t = pers.tile([P, W], bf16, tag=f"wq{c}")
                nc.sync.dma_start(out=t[:], in_=wqT_d[c * P:(c + 1) * P, :])
                wq.append(t)
            for c in range(DC):
                t = pers.tile([P, W], bf16, tag=f"wk{c}")
                nc.sync.dma_start(out=t[:], in_=wkT_d[c * P:(c + 1) * P, :])
                wk.append(t)
            for c in range(DC):
                t = pers.tile([P, W], bf16, tag=f"wv{c}")
                nc.sync.dma_start(out=t[:], in_=wvT_d[c * P:(c + 1) * P, :])
                wv.append(t)
            bq_t = pers.tile([P, DC], f32, tag="bq")
            nc.sync.dma_start(out=bq_t[:], in_=bq_d.rearrange("(n p) -> p n", p=P))
            bk_t = pers.tile([P, DC], f32, tag="bk")
            nc.sync.dma_start(out=bk_t[:], in_=bk_d.rearrange("(n p) -> p n", p=P))
            bvb_t = pers.tile([P, W], f32, tag="bvb")
            nc.sync.dma_start(out=bvb_t[:], in_=bvb_d[:])
            causal = []
            for j in range(W // P):
                t = pers.tile([P, W], bf16, tag=f"causal{j}")
                nc.sync.dma_start(out=t[:], in_=causal_d[j])
                causal.append(t)
            ones_t = pers.tile([P, 1], bf16, tag="ones")
            nc.sync.dma_start(out=ones_t[:], in_=ones_d[:])
            zt = pers.tile([P, W], f32, tag="zt")
            nc.gpsimd.memset(zt[:], 0.0)

            import contextlib
            rep_ctx = (
                tc.For_i(0, repeat, 1) if loop else contextlib.nullcontext(0)
            )
            with rep_ctx:
              for _rep in range(1 if loop else repeat):
                for seq in range(BPC):
                    KB = slot_caps[seq]
                    SCcap = -(-KB // (W // P))
                    kbias_t = mp.tile([P, SB], f32, tag="kbias")
                    nc.sync.dma_start(
                        out=kbias_t[:],
                        in_=kbias_d[seq].rearrange("(n p) -> p n", p=P),
                    )
                    qmask_t = mp.tile([P, SB], f32, tag="qmask")
                    nc.sync.dma_start(
                        out=qmask_t[:],
                        in_=qmask_d[seq].rearrange("(n p) -> p n", p=P),
                    )
                    xt = []
                    for c in range(DC):
                        t = xp.tile([P, S], bf16, tag="xt")
                        nc.sync.dma_start(
                            out=t[:], in_=xT_d[seq, c * P:(c + 1) * P, :]
                        )
                        xt.append(t)

                    # ---- Q/K projections (exact widths up to the cap) ----
                    qT = [qp.tile([P, S], bf16, name="qt", tag="qt")
                          for _ in range(DC)]
                    kT = [kp.tile([P, S], bf16, name="kt", tag="kt")
                          for _ in range(DC)]
                    for db in range(DC):
                        for sc in range(SCcap):
                            w = min(W, KB * P - sc * W)
                            pq = pqs.tile([P, W], f32, tag="pqs")
                            for c in range(DC):
                                nc.tensor.matmul(
                                    pq[:, :w],
                                    wq[c][:, db * P:(db + 1) * P],
                                    xt[c][:, sc * W:sc * W + w],
                                    start=(c == 0),
                                    stop=(c == DC - 1),
                                )
                            nc.scalar.activation(
                                qT[db][:, sc * W:sc * W + w],
                                pq[:, :w],
                                mybir.ActivationFunctionType.Identity,
                                bias=bq_t[:, db:db + 1],
                                scale=1.0,
                            )
                            pk = pqs.tile([P, W], f32, tag="pqs")
                            for c in range(DC):
                                nc.tensor.matmul(
                                    pk[:, :w],
                                    wk[c][:, db * P:(db + 1) * P],
                                    xt[c][:, sc * W:sc * W + w],
                                    start=(c == 0),
                                    stop=(c == DC - 1),
                                )
                            nc.scalar.activation(
                                kT[db][:, sc * W:sc * W + w],
                                pk[:, :w],
                                mybir.ActivationFunctionType.Identity,
                                bias=bk_t[:, db:db + 1],
                                scale=1.0,
                            )

                    # ---- V projection (bias via DVE broadcast add) ----
                    vv = []
                    for sb in range(KB):
                        pv = pqs.tile([P, W], f32, tag="pqs")
                        for c in range(DC):
                            nc.tensor.matmul(
                                pv[:],
                                xt[c][:, sb * P:(sb + 1) * P],
                                wv[c][:],
                                start=(c == 0),
                                stop=(c == DC - 1),
                            )
                        tv = vp.tile([P, W], bf16, tag="vv")
                        nc.vector.tensor_add(tv[:], pv[:], bvb_t[:])
                        vv.append(tv)

                    # ---- attention, one q-chunk (512 queries) at a time ----
                    for qc in range(SCcap):
                        kmax = min((qc + 1) * (W // P), KB)
                        exps = []
                        for kb in range(kmax):
                            ps = pqs.tile([P, W], f32, tag="pqs")
                            for dc in range(DC):
                                nc.tensor.matmul(
                                    ps[:],
                                    kT[dc][:, kb * P:(kb + 1) * P],
                                    qT[dc][:, qc * W:(qc + 1) * W],
                                    start=(dc == 0),
                                    stop=(dc == DC - 1),
                                )
                            et = ep.tile([P, W], bf16, tag="ex")
                            nc.scalar.activation(
                                et[:],
                                ps[:],
                                mybir.ActivationFunctionType.Exp,
                                bias=kbias_t[:, kb:kb + 1],
                                scale=1.0,
                            )
                            j = kb - qc * (W // P)
                            if j >= 0:
                                # diagonal block: zero the k>q triangle with
                                # a 0/1 mask on DVE (bf16, 2x rate)
                                nc.vector.tensor_tensor(
                                    et[:], et[:], causal[j][:],
                                    op=mybir.AluOpType.mult,
                                )
                            exps.append(et)

                        # denominator: bf16 pairwise tree-sum on DVE
                        level = exps
                        while len(level) > 1:
                            nxt = []
                            i = 0
                            while i + 1 < len(level):
                                t = dap.tile([P, W], bf16, tag="da")
                                nc.vector.tensor_add(
                                    t[:], level[i][:], level[i + 1][:]
                                )
                                nxt.append(t)
                                i += 2
                            if i < len(level):
                                nxt.append(level[i])
                            level = nxt
                        dacc = level[0]

                        # out_un[q,d] = sum_k expT[k,q]^T V[k,d]
                        po_list = []
                        for jq in range(W // P):
                            qb = qc * (W // P) + jq
                            if qb >= KB:
                                continue
                            po = pop.tile([P, W], f32, tag="po")
                            for kb in range(qb + 1):
                                nc.tensor.matmul(
                                    po[:],
                                    exps[kb][:, jq * P:(jq + 1) * P],
                                    vv[kb][:],
                                    start=(kb == 0),
                                    stop=(kb == qb),
                                )
                            po_list.append((jq, qb, po))

                        # denom straight into [128q, 4] layout: 4x N=1 mm
                        pd = pdp.tile([P, W // P], f32, tag="pd")
                        for j in range(W // P):
                            nc.tensor.matmul(
                                pd[:, j:j + 1],
                                dacc[:, j * P:(j + 1) * P],
                                ones_t[:],
                                start=True, stop=True,
                            )
                        scl = mp.tile([P, W // P], f32, tag="scl")
                        nc.vector.tensor_scalar_add(scl[:], pd[:], EPS)
                        nc.vector.reciprocal(scl[:], scl[:])
                        nc.vector.tensor_tensor(
                            scl[:],
                            scl[:],
                            qmask_t[:, qc * (W // P):(qc + 1) * (W // P)],
                            op=mybir.AluOpType.mult,
                        )
                        for jq, qb, po in po_list:
                            ot = op_.tile([P, W], f32, tag="outs")
                            nc.vector.tensor_scalar_mul(
                                ot[:], po[:], scl[:, jq:jq + 1]
                            )
                            nc.sync.dma_start(
                                out=out_d[seq, qb * P:(qb + 1) * P, :],
                                in_=ot[:],
                            )

                    # rows in blocks >= KB are entirely padded queries: zero
                    for qb in range(KB, SB):
                        nc.sync.dma_start(
                            out=out_d[seq, qb * P:(qb + 1) * P, :],
                            in_=zt[:],
                        )
    nc.finalize()
    return nc


def prep_inputs(x, Wq, bq, Wk, bk, Wv, bv, padding_mask):
    """Host-side layout prep + sharding. Returns per-core in_maps."""
    x = np.asarray(x, dtype=np.float32)
    pad = np.asarray(padding_mask).astype(bool)
    sc = 1.0 / np.sqrt(np.float32(D))
    wqT = np.ascontiguousarray((np.asarray(Wq, np.float32).T * sc)).astype(BF16)
    wkT = np.ascontiguousarray(np.asarray(Wk, np.float32).T).astype(BF16)
    wvT = np.ascontiguousarray(np.asarray(Wv, np.float32).T).astype(BF16)
    bq_s = (np.asarray(bq, np.float32) * sc).astype(np.float32)
    bk_a = np.asarray(bk, np.float32)
    bvb = np.tile(np.asarray(bv, np.float32).reshape(1, D), (P, 1))
    kbias = np.where(pad, np.float32(NEG), np.float32(0.0)).astype(np.float32)
    qmask = np.where(pad, np.float32(0.0), np.float32(1.0)).astype(np.float32)

    # 0/1 triangular tiles for the 4 diagonal sub-blocks of a
    # [k=128, q=512] scoresT tile: zero where 128*j + k_local > q_local
    kl = np.arange(P)[:, None]
    ql = np.arange(W)[None, :]
    causal = np.stack(
        [np.where(P * j + kl > ql, np.float32(0.0), np.float32(1.0))
         for j in range(W // P)]
    ).astype(BF16)
    ones = np.ones((P, 1), dtype=np.float32).astype(BF16)

    xT = np.ascontiguousarray(x.transpose(0, 2, 1)).astype(BF16)  # [B, D, S]

    # per-seq valid-block cap from the actual mask (exact for any mask)
    valid = ~pad
    caps = np.zeros(B, dtype=np.int64)
    for b in range(B):
        idx = np.nonzero(valid[b])[0]
        caps[b] = 0 if idx.size == 0 else int(np.ceil((idx[-1] + 1) / P))
    order = np.argsort(-caps, kind="stable")  # descending cap
    perm = []
    for i in range(N_CORES):
        perm.extend([int(order[B - 1 - i]), int(order[i])])
    slot_caps = (int(caps[order[N_CORES]]), int(caps[order[0]]))

    in_maps = []
    for i in range(N_CORES):
        sel = [perm[2 * i], perm[2 * i + 1]]
        in_maps.append({
            "xT": np.ascontiguousarray(xT[sel]),
            "wqT": wqT, "wkT": wkT, "wvT": wvT,
            "bq": bq_s, "bk": bk_a, "bvb": bvb,
            "kbias": np.ascontiguousarray(kbias[sel]),
            "qmask": np.ascontiguousarray(qmask[sel]),
            "causal": causal, "ones": ones,
        })
    return in_maps, perm, slot_caps


_NC_CACHE = {}


def get_nc(repeat: int = 1, loop: bool = False, slot_caps=(SB, SB)):
    key = (repeat, loop, slot_caps)
    if key not in _NC_CACHE:
        _NC_CACHE[key] = build_nc(repeat, loop, slot_caps)
    return _NC_CACHE[key]


def kernel(x, Wq, bq, Wk, bk, Wv, bv, padding_mask):
    in_maps, perm, slot_caps = prep_inputs(
        x, Wq, bq, Wk, bk, Wv, bv, padding_mask)
    nc = get_nc(1, slot_caps=slot_caps)
    r = run_bass_kernel_spmd(nc, in_maps, list(range(N_CORES)))
    out = np.empty((B, S, D), dtype=np.float32)
    for j, orig in enumerate(perm):
        out[orig] = r.results[j // BPC]["out"][j % BPC]
    return out
